# revision 16
# baseline (speedup 1.0000x reference)
# Trainium2 Bass kernel for ChunkLlamaAttention (chunked attention w/ 3 rope
# variants + LSE merge), tensor-parallel over 8 NeuronCores.
#
# Sharding: 16 q-heads / 4 kv-heads split as 2 q-heads + 1 kv-head per core.
# Each core: QKV projections (bf16 matmuls, f32 psum) -> fused k-rope ->
# 3 roped copies of q (intra / cross / far) -> unified-softmax chunked
# attention (the reference's per-part LSE merge == one softmax over the
# union of keys, with q roped per key-block's chunk distance) -> o_proj
# partial (columns of attn heads x Wo^T rows). Host sums the 8 partials.
#
# Layouts: q/k kept transposed [head_dim(128 part), seq]; scores computed as
# S^T [k, q] so softmax denom comes from a ones-stationary matmul; PV uses
# v [k, d] stationary giving out^T [d, q] which feeds o_proj stationary
# directly. exp() runs on ACT (scale folded in); probabilities in fp16.
#
# Scheduling (v1 restructure):
#  - DMA order: proj-critical tensors first; rope tables mid proj loop.
#  - v-transposes + per-chunk q-rope interleaved into the proj loop so
#    attention starts right at proj end.
#  - Attention per (c,h,qt): k-blocks paired so one ACT exp covers two
#    score tiles (2-bank [128,2,512] psum tiles); intra diagonal blocks
#    column-trimmed; triangular masks on the (idle) Pool engine; the 4
#    qt softmax denominators accumulate in one psum bank at partition
#    offsets 0/32/64/96; fast approx reciprocal + partition broadcast +
#    fused normalize-copy (psum -> attnT in a single DVE multiply).
#  - o_proj of chunk c interleaved into chunk c+1's attention; tail
#    copies alternate ACT/DVE.
import numpy as np
import ml_dtypes
from contextlib import ExitStack

import concourse.bass as bass
import concourse.mybir as mybir
import concourse.tile as tile
from concourse import bacc
from concourse.bass_utils import run_bass_kernel_spmd
from concourse.masks import make_identity

BF16 = mybir.dt.bfloat16
FP16 = mybir.dt.float16
F32 = mybir.dt.float32
NPBF16 = ml_dtypes.bfloat16

N_CORES = 8
SEQ = 4992
HID = 2048
CL = 1664          # chunk length
NCHUNK = 3
D = 128            # head dim
NH_CORE = 2        # q heads per core
QT = 416           # q tile (4 per chunk)
NQT = 4
NKB = CL // 128    # 13 k-blocks per chunk
HC = HID // 128    # 16 hidden chunks
NSB = SEQ // 128   # 39 s-blocks
SCALE = float(D) ** -0.5


def _build():
    nc = bacc.Bacc("TRN2", target_bir_lowering=False, debug=False,
                   num_devices=N_CORES)
    hT = nc.dram_tensor("hT", [HID, SEQ], BF16, kind="ExternalInput").ap()
    wq = nc.dram_tensor("wq", [HID, NH_CORE * D], BF16, kind="ExternalInput").ap()
    wk = nc.dram_tensor("wk", [HID, D], BF16, kind="ExternalInput").ap()
    wv = nc.dram_tensor("wv", [HID, D], BF16, kind="ExternalInput").ap()
    wo = nc.dram_tensor("wo", [NH_CORE * D, HID], BF16, kind="ExternalInput").ap()
    cosI = nc.dram_tensor("cosI", [D, SEQ], BF16, kind="ExternalInput").ap()
    sinIS = nc.dram_tensor("sinIS", [D, SEQ], BF16, kind="ExternalInput").ap()
    cosC = nc.dram_tensor("cosC", [D, 2 * CL], BF16, kind="ExternalInput").ap()
    sinCS = nc.dram_tensor("sinCS", [D, 2 * CL], BF16, kind="ExternalInput").ap()
    cosF = nc.dram_tensor("cosF", [D, 1], F32, kind="ExternalInput").ap()
    sinFS = nc.dram_tensor("sinFS", [D, 1], F32, kind="ExternalInput").ap()
    cosK = nc.dram_tensor("cosK", [D, SEQ], BF16, kind="ExternalInput").ap()
    sinKS = nc.dram_tensor("sinKS", [D, SEQ], BF16, kind="ExternalInput").ap()
    bigtri_in = nc.dram_tensor("bigtri", [D, 928], FP16, kind="ExternalInput").ap()
    o_out = nc.dram_tensor("o_out", [SEQ, HID], BF16, kind="ExternalOutput").ap()

    with tile.TileContext(nc) as tc, ExitStack() as ctx:
        persist = ctx.enter_context(tc.tile_pool(name="persist", bufs=1))
        ones = persist.tile([128, 1], FP16)
        nc.gpsimd.memset(ones[:], 1.0)
        ident = persist.tile([128, 128], FP16)
        make_identity(nc, ident[:])

        # proj-critical DMAs first: weight slices, then k-rope tables
        wq_sb = persist.tile([128, HC, NH_CORE * D], BF16)
        nc.sync.dma_start(wq_sb[:], wq.rearrange("(hc p) d -> p hc d", p=128))
        wk_sb = persist.tile([128, HC, D], BF16)
        nc.sync.dma_start(wk_sb[:], wk.rearrange("(hc p) d -> p hc d", p=128))
        wv_sb = persist.tile([128, HC, D], BF16)
        nc.sync.dma_start(wv_sb[:], wv.rearrange("(hc p) d -> p hc d", p=128))

        kT = persist.tile([128, SEQ], BF16)          # roped keys [d, s]
        v_sb = persist.tile([128, NSB, 128], FP16)   # [s_in_blk, blk, d]
        attnT = [persist.tile([128, SEQ], BF16, name=f"attnT{h}")
                 for h in range(NH_CORE)]
        wo_sb = persist.tile([128, NH_CORE, HID], BF16)
        bigtri = persist.tile([128, 928], FP16)
        cosF_sb = persist.tile([128, 1], F32)
        sinFS_sb = persist.tile([128, 1], F32)

        qraw_pool = ctx.enter_context(tc.tile_pool(name="qraw", bufs=1))
        qraw = [qraw_pool.tile([128, SEQ], BF16, name=f"qraw{h}")
                for h in range(NH_CORE)]
        rp = ctx.enter_context(tc.tile_pool(name="ropesb", bufs=1))
        ct = rp.tile([128, SEQ], BF16, tag="ct", name="cosI_sb")
        st_t = rp.tile([128, SEQ], BF16, tag="st", name="sinIS_sb")
        ct2 = rp.tile([128, 2 * CL], BF16, tag="ct2", name="cosC_sb")
        st2 = rp.tile([128, 2 * CL], BF16, tag="st2", name="sinCS_sb")

        qsets = ctx.enter_context(tc.tile_pool(name="qsets", bufs=1))
        qint = [qsets.tile([128, SEQ], BF16, name=f"qint{h}")
                for h in range(NH_CORE)]
        qcrs = [qsets.tile([128, 2 * CL], BF16, name=f"qcrs{h}")
                for h in range(NH_CORE)]
        qfar = [qsets.tile([128, CL], BF16, name=f"qfar{h}")
                for h in range(NH_CORE)]

        def rope_block(dst, src_ap, ct_ap, st_ap, L, nm):
            # dst = src*cos + rot_half(src)*sin_signed, all on DVE
            m = rp.tile([128, CL], BF16, tag="ropem", bufs=1, name=f"m{nm}")
            r = rp.tile([128, CL], BF16, tag="roper", bufs=1, name=f"r{nm}")
            t = rp.tile([128, CL], BF16, tag="ropet", bufs=1, name=f"t{nm}")
            nc.vector.tensor_copy(r[0:64, 0:L], src_ap[64:128])
            nc.vector.tensor_copy(r[64:128, 0:L], src_ap[0:64])
            nc.vector.tensor_mul(m[:, 0:L], src_ap, ct_ap)
            nc.vector.tensor_mul(t[:, 0:L], r[:, 0:L], st_ap)
            nc.vector.tensor_add(dst, m[:, 0:L], t[:, 0:L])

        def rope_chunk(c):
            # intra rope for chunk c; cross for c>=1; far for c==2
            a, b = c * CL, (c + 1) * CL
            for h in range(NH_CORE):
                rope_block(qint[h][:, a:b], qraw[h][:, a:b], ct[:, a:b],
                           st_t[:, a:b], CL, f"i{h}{c}")
            if c >= 1:
                ca, cb = (c - 1) * CL, c * CL
                for h in range(NH_CORE):
                    rope_block(qcrs[h][:, ca:cb], qraw[h][:, a:b],
                               ct2[:, ca:cb], st2[:, ca:cb], CL, f"c{h}{c}")
            if c == 2:
                for h in range(NH_CORE):
                    m = rp.tile([128, CL], BF16, tag="ropem", bufs=1,
                                name=f"mf{h}")
                    r = rp.tile([128, CL], BF16, tag="roper", bufs=1,
                                name=f"rf{h}")
                    nc.vector.tensor_copy(r[0:64, :], qraw[h][64:128, a:b])
                    nc.vector.tensor_copy(r[64:128, :], qraw[h][0:64, a:b])
                    nc.vector.tensor_scalar_mul(m[:], qraw[h][:, a:b],
                                                cosF_sb[:])
                    nc.vector.scalar_tensor_tensor(
                        qfar[h][:, :], r[:], sinFS_sb[:], m[:],
                        op0=mybir.AluOpType.mult, op1=mybir.AluOpType.add)

        # ---------- Phase 1: QKV projections + fused k-rope, interleaved
        # with v-transpose per s-tile, q-rope per finished chunk, and the
        # deferred DMAs ----------
        with tc.tile_pool(name="projsb", bufs=1) as pj, \
             tc.tile_pool(name="projpsum", bufs=1, space="PSUM") as pp, \
             tc.tile_pool(name="tpsum", bufs=2, space="PSUM") as tp:
            cosK_sb = pj.tile([128, SEQ], BF16)
            nc.sync.dma_start(cosK_sb[:], cosK[:])
            sinKS_sb = pj.tile([128, SEQ], BF16)
            nc.sync.dma_start(sinKS_sb[:], sinKS[:])
            n_st = (SEQ + 511) // 512
            for st in range(n_st):
                s0 = st * 512
                L = min(512, SEQ - s0)
                hts = []
                for hc in range(HC):
                    ht_t = pj.tile([128, 512], BF16, tag="htile", bufs=16,
                                   name=f"ht_{st}_{hc}")
                    nc.sync.dma_start(ht_t[:, 0:L], hT[hc * 128:(hc + 1) * 128,
                                                       s0:s0 + L])
                    hts.append(ht_t)
                pq0 = pp.tile([128, 512], F32, tag="pq0")
                pq1 = pp.tile([128, 512], F32, tag="pq1")
                pk = pp.tile([128, 512], F32, tag="pk", bufs=2)
                pv = pp.tile([128, 512], F32, tag="pv")
                for hc in range(HC):
                    fst = hc == 0
                    lst = hc == HC - 1
                    rhs = hts[hc][:, 0:L]
                    nc.tensor.matmul(pq0[:, 0:L], wq_sb[:, hc, 0:128], rhs,
                                     start=fst, stop=lst)
                    nc.tensor.matmul(pq1[:, 0:L], wq_sb[:, hc, 128:256], rhs,
                                     start=fst, stop=lst)
                    nc.tensor.matmul(pk[:, 0:L], wk_sb[:, hc, :], rhs,
                                     start=fst, stop=lst)
                    nc.tensor.matmul(pv[:, 0:L], wv_sb[:, hc, :], rhs,
                                     start=fst, stop=lst)
                nc.vector.tensor_copy(qraw[0][:, s0:s0 + L], pq0[:, 0:L])
                nc.vector.tensor_copy(qraw[1][:, s0:s0 + L], pq1[:, 0:L])
                vT = pj.tile([128, 512], FP16, tag="vT", bufs=2,
                             name=f"vT{st}")
                nc.scalar.copy(vT[:, 0:L], pv[:, 0:L])
                # fused k rope: kT = pk*cosK + rot(pk)*sinKS
                # (walrus requires equal input base partitions for 2-input
                # DVE ops, so stage the half-swap via 1-input copies)
                mt = pj.tile([128, 512], BF16, tag="mt", bufs=1)
                rt = pj.tile([128, 512], BF16, tag="rt", bufs=1)
                tt = pj.tile([128, 512], BF16, tag="tt", bufs=1)
                nc.vector.tensor_copy(rt[0:64, 0:L], pk[64:128, 0:L])
                nc.vector.tensor_copy(rt[64:128, 0:L], pk[0:64, 0:L])
                nc.vector.tensor_mul(mt[:, 0:L], pk[:, 0:L],
                                     cosK_sb[:, s0:s0 + L])
                nc.vector.tensor_mul(tt[:, 0:L], rt[:, 0:L],
                                     sinKS_sb[:, s0:s0 + L])
                nc.vector.tensor_add(kT[:, s0:s0 + L], mt[:, 0:L], tt[:, 0:L])
                # v transpose for the s-blocks this tile covers
                for sb in range(s0 // 128, (s0 + L) // 128):
                    w0 = sb * 128 - s0
                    ptr = tp.tile([128, 128], FP16, tag="ptr")
                    nc.tensor.transpose(ptr[:], vT[:, w0:w0 + 128], ident[:])
                    nc.scalar.copy(v_sb[:, sb, :], ptr[:])
                # deferred DMAs + per-chunk rope, injected at milestones
                if st == 0:
                    nc.sync.dma_start(ct[:], cosI[:])
                    nc.sync.dma_start(st_t[:], sinIS[:])
                elif st == 1:
                    nc.sync.dma_start(ct2[:], cosC[:])
                    nc.sync.dma_start(st2[:], sinCS[:])
                elif st == 2:
                    nc.sync.dma_start(bigtri[:], bigtri_in[:])
                    nc.sync.dma_start(cosF_sb[:], cosF[:])
                    nc.sync.dma_start(sinFS_sb[:], sinFS[:])
                    nc.sync.dma_start(
                        wo_sb[:], wo.rearrange("(fc p) h -> p fc h", p=128))
                elif st == 3:
                    rope_chunk(0)
                elif st == 6:
                    rope_chunk(1)
                elif st == 9:
                    rope_chunk(2)

        # ---------- Phase 2: attention per chunk + interleaved o_proj ----
        # Per (c, h, qt): iterate k-blocks (intra column-trimmed), paired so
        # one exp covers two score tiles. PV accumulates pos [d, qt]; the
        # denominator z accumulates in pzs partition row 32*qt. Then fast
        # reciprocal -> broadcast -> fused normalize into attnT.
        oproj_q = []          # pending o_proj units from the previous chunk

        def emit_oproj(sb, ht, engine):
            pp_ = ap_.tile([128, 512], F32, tag="po_", bufs=2,
                           name=f"pp{sb}{ht}")
            for fc in range(NH_CORE):
                nc.tensor.matmul(
                    pp_[:],
                    attnT[fc][:, sb * 128:(sb + 1) * 128],
                    wo_sb[:, fc, ht * 512:(ht + 1) * 512],
                    start=(fc == 0), stop=(fc == NH_CORE - 1))
            ob = asb.tile([128, 512], BF16, tag="ob", bufs=4,
                          name=f"ob{sb}{ht}")
            if engine == "v":
                nc.vector.tensor_copy(ob[:], pp_[:])
            else:
                nc.scalar.copy(ob[:], pp_[:])
            nc.sync.dma_start(
                o_out[sb * 128:(sb + 1) * 128,
                      ht * 512:(ht + 1) * 512], ob[:])

        def drain_oproj(k, engine="v"):
            for _ in range(min(k, len(oproj_q))):
                sb, ht = oproj_q.pop(0)
                emit_oproj(sb, ht, engine)

        with tc.tile_pool(name="attnsb", bufs=1) as asb, \
             tc.tile_pool(name="attnpsum", bufs=1, space="PSUM") as ap_:
            for c in range(NCHUNK):
                for h in range(NH_CORE):
                    for qt in range(NQT):
                        q0 = qt * QT
                        # z accumulator: one psum bank reused serially across
                        # qt (matmul psum writes at partition offsets != 0
                        # miscompute on HW, so everything lives at row 0)
                        pzs = ap_.tile([1, 512], F32, tag="z", bufs=1,
                                       name=f"pz{c}{h}{qt}")
                        # item: (qsrc_ap, ka, off, mask(moff, mw, bstart))
                        items = []
                        partials = []
                        nkb_i = min((q0 + QT + 127) // 128, NKB)
                        for kb in range(nkb_i):
                            off = max(0, kb * 128 - q0)
                            delta = q0 - kb * 128
                            mask = None
                            if delta < 128:
                                mw = min(128, QT - off)
                                mask = (off, mw, 512 + delta + off)
                            it = (qint[h][:, c * CL + q0:c * CL + q0 + QT],
                                  c * CL + kb * 128, off, mask)
                            (partials if off else items).append(it)
                        if c >= 1:
                            qs = qcrs[h][:, (c - 1) * CL + q0:
                                         (c - 1) * CL + q0 + QT]
                            for kb in range(NKB):
                                items.append((qs, (c - 1) * CL + kb * 128,
                                              0, None))
                        if c == 2:
                            qs = qfar[h][:, q0:q0 + QT]
                            for kb in range(NKB):
                                items.append((qs, kb * 128, 0, None))
                        items += partials
                        # pair consecutive full-width items
                        groups = []
                        i = 0
                        while i < len(items):
                            if (i + 1 < len(items) and items[i][2] == 0
                                    and items[i + 1][2] == 0):
                                groups.append((items[i], items[i + 1]))
                                i += 2
                            else:
                                groups.append((items[i],))
                                i += 1
                        npv = len(items)
                        pos_t = ap_.tile([128, 512], F32, tag="o", bufs=1,
                                         name=f"po{c}{h}{qt}")
                        cnt = 0
                        for gi, grp in enumerate(groups):
                            ps_t = ap_.tile([128, 2, 512], F32, tag="s",
                                            bufs=2, name=f"ps{c}{h}{qt}_{gi}")
                            PT = asb.tile([128, 2, QT], FP16, tag="PT",
                                          bufs=3, name=f"PT{c}{h}{qt}_{gi}")
                            for sl, (qs, ka, off, mask) in enumerate(grp):
                                nc.tensor.matmul(ps_t[:, sl, off:QT],
                                                 kT[:, ka:ka + 128],
                                                 qs[:, off:QT],
                                                 start=True, stop=True)
                            if len(grp) == 2:
                                nc.scalar.activation(
                                    PT[:, :, :], ps_t[:, :, 0:QT],
                                    mybir.ActivationFunctionType.Exp,
                                    scale=SCALE)
                            else:
                                off = grp[0][2]
                                nc.scalar.activation(
                                    PT[:, 0, off:QT], ps_t[:, 0, off:QT],
                                    mybir.ActivationFunctionType.Exp,
                                    scale=SCALE)
                            for sl, (qs, ka, off, mask) in enumerate(grp):
                                if mask is not None:
                                    moff, mw, bs = mask
                                    nc.gpsimd.tensor_mul(
                                        PT[:, sl, moff:moff + mw],
                                        PT[:, sl, moff:moff + mw],
                                        bigtri[:, bs:bs + mw])
                            for sl, (qs, ka, off, mask) in enumerate(grp):
                                cnt += 1
                                fst = cnt == 1
                                lst = cnt == npv
                                nc.tensor.matmul(pos_t[:, off:QT],
                                                 v_sb[:, ka // 128, :],
                                                 PT[:, sl, off:QT],
                                                 start=fst, stop=lst)
                                nc.tensor.matmul(
                                    pzs[0:1, off:QT],
                                    ones[:], PT[:, sl, off:QT],
                                    start=fst, stop=lst)
                        # normalize: rz = 1/z, broadcast, fused mul into attnT
                        rzf = asb.tile([1, QT], F32, tag="rzf", bufs=2,
                                       name=f"rzf{c}{h}{qt}")
                        nc.vector.reciprocal_approx_fast(
                            rzf[0:1, :], pzs[0:1, 0:QT])
                        rb = asb.tile([128, QT], F32, tag="rb", bufs=2,
                                      name=f"rb{c}{h}{qt}")
                        nc.gpsimd.partition_broadcast(rb[:], rzf[0:1, :])
                        nc.vector.tensor_mul(
                            attnT[h][:, c * CL + q0:c * CL + q0 + QT],
                            pos_t[:, 0:QT], rb[:])
                        drain_oproj(7)
                # chunk done for both heads: queue its o_proj units
                for sb in range(c * NKB, (c + 1) * NKB):
                    for ht in range(4):
                        oproj_q.append((sb, ht))
            # tail flush: alternate copy engine to split ACT/DVE load
            eng = 0
            while oproj_q:
                drain_oproj(1, "v" if eng % 2 == 0 else "s")
                eng += 1
    nc.compile()
    return nc


def _sflip(sT):
    out = np.array(sT, dtype=np.float32)
    out[0:64] = -out[0:64]
    return out


def _prep_in_maps(inputs):
    f32 = np.float32
    hs = np.asarray(inputs["hidden_states"], f32).reshape(SEQ, HID)
    pos = np.asarray(inputs["position_ids"]).reshape(SEQ).astype(np.int64)
    pid = pos % CL
    q_cos = np.asarray(inputs["q_cos"], f32)
    q_sin = np.asarray(inputs["q_sin"], f32)
    qc_cos = np.asarray(inputs["qc_cos"], f32)
    qc_sin = np.asarray(inputs["qc_sin"], f32)
    k_cos = np.asarray(inputs["k_cos"], f32)
    k_sin = np.asarray(inputs["k_sin"], f32)
    Wq = np.asarray(inputs["Wq"], f32)
    Wk = np.asarray(inputs["Wk"], f32)
    Wv = np.asarray(inputs["Wv"], f32)
    Wo = np.asarray(inputs["Wo"], f32)

    hT = np.ascontiguousarray(hs.T).astype(NPBF16)
    cosI = np.ascontiguousarray(q_cos[pid].T).astype(NPBF16)
    sinIS = _sflip(q_sin[pid].T).astype(NPBF16)
    # cross tables for chunks 1..2 (columns (c-1)*CL..c*CL map to chunk c)
    cosC = np.ascontiguousarray(qc_cos[pid[CL:3 * CL]].T).astype(NPBF16)
    sinCS = _sflip(qc_sin[pid[CL:3 * CL]].T).astype(NPBF16)
    cosF = np.ascontiguousarray(qc_cos[CL - 1][:, None]).astype(f32)
    sinFS = _sflip(qc_sin[CL - 1][:, None]).astype(f32)
    cosK = np.ascontiguousarray(k_cos[pos].T).astype(NPBF16)
    sinKS = _sflip(k_sin[pos].T).astype(NPBF16)
    bigtri = (np.arange(128)[:, None] <= (np.arange(928)[None, :] - 512)
              ).astype(np.float16)

    shared = dict(hT=hT, cosI=cosI, sinIS=sinIS, cosC=cosC, sinCS=sinCS,
                  cosF=cosF, sinFS=sinFS, cosK=cosK, sinKS=sinKS,
                  bigtri=bigtri)
    in_maps = []
    for core in range(N_CORES):
        kv = core // 2
        m = dict(shared)
        m["wq"] = np.ascontiguousarray(
            Wq[256 * core:256 * (core + 1), :].T).astype(NPBF16)
        m["wk"] = np.ascontiguousarray(
            Wk[128 * kv:128 * (kv + 1), :].T).astype(NPBF16)
        m["wv"] = np.ascontiguousarray(
            Wv[128 * kv:128 * (kv + 1), :].T).astype(NPBF16)
        m["wo"] = np.ascontiguousarray(
            Wo[:, 256 * core:256 * (core + 1)].T).astype(NPBF16)
        in_maps.append(m)
    return in_maps


_CACHE = {}


def _get_nc():
    if "nc" not in _CACHE:
        _CACHE["nc"] = _build()
    return _CACHE["nc"]


def kernel(**inputs):
    nc = _get_nc()
    in_maps = _prep_in_maps(inputs)
    res = run_bass_kernel_spmd(nc, in_maps, list(range(N_CORES)))
    out = np.zeros((SEQ, HID), np.float32)
    for r in res.results:
        out += r["o_out"].astype(np.float32)
    return out.reshape(1, SEQ, HID).astype(np.float32)


# revision 17
# speedup vs baseline: 1.1407x; 1.1407x over previous
# Trainium2 Bass kernel for ChunkLlamaAttention (chunked attention w/ 3 rope
# variants + LSE merge), tensor-parallel over 8 NeuronCores.
#
# Sharding: 16 q-heads / 4 kv-heads split as 2 q-heads + 1 kv-head per core.
# Each core: QKV projections (bf16 matmuls, f32 psum) -> fused k-rope ->
# 3 roped copies of q (intra / cross / far) -> unified-softmax chunked
# attention (the reference's per-part LSE merge == one softmax over the
# union of keys, with q roped per key-block's chunk distance) -> o_proj
# partial (columns of attn heads x Wo^T rows). Host sums the 8 partials.
#
# Layouts: q/k kept transposed [head_dim(128 part), seq]; scores computed as
# S^T [k, q] so softmax denom comes from a ones-stationary matmul; PV uses
# v [k, d] stationary giving out^T [d, q] which feeds o_proj stationary
# directly. exp() runs on ACT (scale folded in); probabilities in fp16.
#
# Scheduling (v1 restructure):
#  - DMA order: proj-critical tensors first; rope tables mid proj loop.
#  - v-transposes + per-chunk q-rope interleaved into the proj loop so
#    attention starts right at proj end.
#  - Attention per (c,h,qt): k-blocks paired so one ACT exp covers two
#    score tiles (2-bank [128,2,512] psum tiles); intra diagonal blocks
#    column-trimmed; triangular masks on the (idle) Pool engine; the 4
#    qt softmax denominators accumulate in one psum bank at partition
#    offsets 0/32/64/96; fast approx reciprocal + partition broadcast +
#    fused normalize-copy (psum -> attnT in a single DVE multiply).
#  - o_proj of chunk c interleaved into chunk c+1's attention; tail
#    copies alternate ACT/DVE.
import numpy as np
import ml_dtypes
from contextlib import ExitStack

import concourse.bass as bass
import concourse.mybir as mybir
import concourse.tile as tile
from concourse import bacc
from concourse.bass_utils import run_bass_kernel_spmd
from concourse.masks import make_identity

BF16 = mybir.dt.bfloat16
FP16 = mybir.dt.float16
F32 = mybir.dt.float32
NPBF16 = ml_dtypes.bfloat16

N_CORES = 8
SEQ = 4992
HID = 2048
CL = 1664          # chunk length
NCHUNK = 3
D = 128            # head dim
NH_CORE = 2        # q heads per core
QT = 416           # q tile (4 per chunk)
NQT = 4
NKB = CL // 128    # 13 k-blocks per chunk
HC = HID // 128    # 16 hidden chunks
NSB = SEQ // 128   # 39 s-blocks
SCALE = float(D) ** -0.5


def _build():
    nc = bacc.Bacc("TRN2", target_bir_lowering=False, debug=False,
                   num_devices=N_CORES)
    hT = nc.dram_tensor("hT", [HID, SEQ], BF16, kind="ExternalInput").ap()
    wq = nc.dram_tensor("wq", [HID, NH_CORE * D], BF16, kind="ExternalInput").ap()
    wk = nc.dram_tensor("wk", [HID, D], BF16, kind="ExternalInput").ap()
    wv = nc.dram_tensor("wv", [HID, D], BF16, kind="ExternalInput").ap()
    wo = nc.dram_tensor("wo", [NH_CORE * D, HID], BF16, kind="ExternalInput").ap()
    cosI = nc.dram_tensor("cosI", [D, SEQ], BF16, kind="ExternalInput").ap()
    sinIS = nc.dram_tensor("sinIS", [D, SEQ], BF16, kind="ExternalInput").ap()
    cosC = nc.dram_tensor("cosC", [D, 2 * CL], BF16, kind="ExternalInput").ap()
    sinCS = nc.dram_tensor("sinCS", [D, 2 * CL], BF16, kind="ExternalInput").ap()
    cosF = nc.dram_tensor("cosF", [D, 1], F32, kind="ExternalInput").ap()
    sinFS = nc.dram_tensor("sinFS", [D, 1], F32, kind="ExternalInput").ap()
    cosK = nc.dram_tensor("cosK", [D, SEQ], BF16, kind="ExternalInput").ap()
    sinKS = nc.dram_tensor("sinKS", [D, SEQ], BF16, kind="ExternalInput").ap()
    bigtri_in = nc.dram_tensor("bigtri", [D, 928], FP16, kind="ExternalInput").ap()
    o_out = nc.dram_tensor("o_out", [SEQ, HID], BF16, kind="ExternalOutput").ap()

    with tile.TileContext(nc) as tc, ExitStack() as ctx:
        persist = ctx.enter_context(tc.tile_pool(name="persist", bufs=1))
        ones = persist.tile([128, 1], FP16)
        nc.gpsimd.memset(ones[:], 1.0)
        ident = persist.tile([128, 128], FP16)
        make_identity(nc, ident[:])

        # proj-critical DMAs first: weight slices, then k-rope tables
        wq_sb = persist.tile([128, HC, NH_CORE * D], BF16)
        nc.sync.dma_start(wq_sb[:], wq.rearrange("(hc p) d -> p hc d", p=128))
        wk_sb = persist.tile([128, HC, D], BF16)
        nc.sync.dma_start(wk_sb[:], wk.rearrange("(hc p) d -> p hc d", p=128))
        wv_sb = persist.tile([128, HC, D], BF16)
        nc.sync.dma_start(wv_sb[:], wv.rearrange("(hc p) d -> p hc d", p=128))

        kT = persist.tile([128, SEQ], BF16)          # roped keys [d, s]
        v_sb = persist.tile([128, NSB, 128], FP16)   # [s_in_blk, blk, d]
        attnT = [persist.tile([128, SEQ], BF16, name=f"attnT{h}")
                 for h in range(NH_CORE)]
        wo_sb = persist.tile([128, NH_CORE, HID], BF16)
        bigtri = persist.tile([128, 928], FP16)
        cosF_sb = persist.tile([128, 1], F32)
        sinFS_sb = persist.tile([128, 1], F32)

        qraw_pool = ctx.enter_context(tc.tile_pool(name="qraw", bufs=1))
        qraw = [qraw_pool.tile([128, SEQ], BF16, name=f"qraw{h}")
                for h in range(NH_CORE)]
        rp = ctx.enter_context(tc.tile_pool(name="ropesb", bufs=1))
        ct = rp.tile([128, SEQ], BF16, tag="ct", name="cosI_sb")
        st_t = rp.tile([128, SEQ], BF16, tag="st", name="sinIS_sb")
        ct2 = rp.tile([128, 2 * CL], BF16, tag="ct2", name="cosC_sb")
        st2 = rp.tile([128, 2 * CL], BF16, tag="st2", name="sinCS_sb")

        qsets = ctx.enter_context(tc.tile_pool(name="qsets", bufs=1))
        qint = [qsets.tile([128, SEQ], BF16, name=f"qint{h}")
                for h in range(NH_CORE)]
        qcrs = [qsets.tile([128, 2 * CL], BF16, name=f"qcrs{h}")
                for h in range(NH_CORE)]
        qfar = [qsets.tile([128, CL], BF16, name=f"qfar{h}")
                for h in range(NH_CORE)]

        def rope_block(dst, src_ap, ct_ap, st_ap, L, nm):
            # dst = src*cos + rot_half(src)*sin_signed, all on DVE
            m = rp.tile([128, CL], BF16, tag="ropem", bufs=1, name=f"m{nm}")
            r = rp.tile([128, CL], BF16, tag="roper", bufs=1, name=f"r{nm}")
            t = rp.tile([128, CL], BF16, tag="ropet", bufs=1, name=f"t{nm}")
            nc.vector.tensor_copy(r[0:64, 0:L], src_ap[64:128])
            nc.vector.tensor_copy(r[64:128, 0:L], src_ap[0:64])
            nc.vector.tensor_mul(m[:, 0:L], src_ap, ct_ap)
            nc.vector.tensor_mul(t[:, 0:L], r[:, 0:L], st_ap)
            nc.vector.tensor_add(dst, m[:, 0:L], t[:, 0:L])

        def rope_chunk(c):
            # intra rope for chunk c; cross for c>=1; far for c==2
            a, b = c * CL, (c + 1) * CL
            for h in range(NH_CORE):
                rope_block(qint[h][:, a:b], qraw[h][:, a:b], ct[:, a:b],
                           st_t[:, a:b], CL, f"i{h}{c}")
            if c >= 1:
                ca, cb = (c - 1) * CL, c * CL
                for h in range(NH_CORE):
                    rope_block(qcrs[h][:, ca:cb], qraw[h][:, a:b],
                               ct2[:, ca:cb], st2[:, ca:cb], CL, f"c{h}{c}")
            if c == 2:
                for h in range(NH_CORE):
                    m = rp.tile([128, CL], BF16, tag="ropem", bufs=1,
                                name=f"mf{h}")
                    r = rp.tile([128, CL], BF16, tag="roper", bufs=1,
                                name=f"rf{h}")
                    nc.vector.tensor_copy(r[0:64, :], qraw[h][64:128, a:b])
                    nc.vector.tensor_copy(r[64:128, :], qraw[h][0:64, a:b])
                    nc.vector.tensor_scalar_mul(m[:], qraw[h][:, a:b],
                                                cosF_sb[:])
                    nc.vector.scalar_tensor_tensor(
                        qfar[h][:, :], r[:], sinFS_sb[:], m[:],
                        op0=mybir.AluOpType.mult, op1=mybir.AluOpType.add)

        # ---------- Phase 1: QKV projections + fused k-rope, interleaved
        # with v-transpose per s-tile, q-rope per finished chunk, and the
        # deferred DMAs ----------
        with tc.tile_pool(name="projsb", bufs=1) as pj, \
             tc.tile_pool(name="projpsum", bufs=1, space="PSUM") as pp, \
             tc.tile_pool(name="tpsum", bufs=2, space="PSUM") as tp:
            cosK_sb = pj.tile([128, SEQ], BF16)
            nc.sync.dma_start(cosK_sb[:], cosK[:])
            sinKS_sb = pj.tile([128, SEQ], BF16)
            nc.sync.dma_start(sinKS_sb[:], sinKS[:])
            n_st = (SEQ + 511) // 512
            for st in range(n_st):
                s0 = st * 512
                L = min(512, SEQ - s0)
                hts = []
                for hc in range(HC):
                    ht_t = pj.tile([128, 512], BF16, tag="htile", bufs=16,
                                   name=f"ht_{st}_{hc}")
                    nc.sync.dma_start(ht_t[:, 0:L], hT[hc * 128:(hc + 1) * 128,
                                                       s0:s0 + L])
                    hts.append(ht_t)
                pq0 = pp.tile([128, 512], F32, tag="pq0")
                pq1 = pp.tile([128, 512], F32, tag="pq1")
                pk = pp.tile([128, 512], F32, tag="pk", bufs=2)
                pv = pp.tile([128, 512], F32, tag="pv")
                for hc in range(HC):
                    fst = hc == 0
                    lst = hc == HC - 1
                    rhs = hts[hc][:, 0:L]
                    nc.tensor.matmul(pq0[:, 0:L], wq_sb[:, hc, 0:128], rhs,
                                     start=fst, stop=lst)
                    nc.tensor.matmul(pq1[:, 0:L], wq_sb[:, hc, 128:256], rhs,
                                     start=fst, stop=lst)
                    nc.tensor.matmul(pk[:, 0:L], wk_sb[:, hc, :], rhs,
                                     start=fst, stop=lst)
                    nc.tensor.matmul(pv[:, 0:L], wv_sb[:, hc, :], rhs,
                                     start=fst, stop=lst)
                nc.vector.tensor_copy(qraw[0][:, s0:s0 + L], pq0[:, 0:L])
                nc.vector.tensor_copy(qraw[1][:, s0:s0 + L], pq1[:, 0:L])
                vT = pj.tile([128, 512], FP16, tag="vT", bufs=2,
                             name=f"vT{st}")
                nc.scalar.copy(vT[:, 0:L], pv[:, 0:L])
                # fused k rope: kT = pk*cosK + rot(pk)*sinKS
                # (walrus requires equal input base partitions for 2-input
                # DVE ops, so stage the half-swap via 1-input copies)
                mt = pj.tile([128, 512], BF16, tag="mt", bufs=1)
                rt = pj.tile([128, 512], BF16, tag="rt", bufs=1)
                tt = pj.tile([128, 512], BF16, tag="tt", bufs=1)
                nc.vector.tensor_copy(rt[0:64, 0:L], pk[64:128, 0:L])
                nc.vector.tensor_copy(rt[64:128, 0:L], pk[0:64, 0:L])
                nc.vector.tensor_mul(mt[:, 0:L], pk[:, 0:L],
                                     cosK_sb[:, s0:s0 + L])
                nc.vector.tensor_mul(tt[:, 0:L], rt[:, 0:L],
                                     sinKS_sb[:, s0:s0 + L])
                nc.vector.tensor_add(kT[:, s0:s0 + L], mt[:, 0:L], tt[:, 0:L])
                # v transpose for the s-blocks this tile covers
                for sb in range(s0 // 128, (s0 + L) // 128):
                    w0 = sb * 128 - s0
                    ptr = tp.tile([128, 128], FP16, tag="ptr")
                    nc.tensor.transpose(ptr[:], vT[:, w0:w0 + 128], ident[:])
                    nc.scalar.copy(v_sb[:, sb, :], ptr[:])
                # deferred DMAs + per-chunk rope, injected at milestones
                if st == 0:
                    nc.sync.dma_start(ct[:], cosI[:])
                    nc.sync.dma_start(st_t[:], sinIS[:])
                elif st == 1:
                    nc.sync.dma_start(ct2[:], cosC[:])
                    nc.sync.dma_start(st2[:], sinCS[:])
                elif st == 2:
                    nc.sync.dma_start(bigtri[:], bigtri_in[:])
                    nc.sync.dma_start(cosF_sb[:], cosF[:])
                    nc.sync.dma_start(sinFS_sb[:], sinFS[:])
                    nc.sync.dma_start(
                        wo_sb[:], wo.rearrange("(fc p) h -> p fc h", p=128))
                elif st == 3:
                    rope_chunk(0)
                elif st == 6:
                    rope_chunk(1)
                elif st == 9:
                    rope_chunk(2)

        # ---------- Phase 2: attention per chunk + interleaved o_proj ----
        # Per (c, h, qt): iterate k-blocks (intra column-trimmed), paired so
        # one exp covers two score tiles. PV accumulates pos [d, qt]; the
        # denominator z accumulates in pzs partition row 32*qt. Then fast
        # reciprocal -> broadcast -> fused normalize into attnT.
        oproj_q = []          # pending o_proj units from the previous chunk
        eng_alt = [0]

        def emit_oproj(pool, bufs, sb, ht):
            pp_ = pool.tile([128, 512], F32, tag="po_", bufs=bufs,
                            name=f"pp{sb}{ht}")
            for fc in range(NH_CORE):
                nc.tensor.matmul(
                    pp_[:],
                    attnT[fc][:, sb * 128:(sb + 1) * 128],
                    wo_sb[:, fc, ht * 512:(ht + 1) * 512],
                    start=(fc == 0), stop=(fc == NH_CORE - 1))
            ob = asb.tile([128, 512], BF16, tag="ob", bufs=4,
                          name=f"ob{sb}{ht}")
            eng_alt[0] += 1
            if eng_alt[0] % 2 == 0:
                nc.vector.tensor_copy(ob[:], pp_[:])
            else:
                nc.scalar.copy(ob[:], pp_[:])
            nc.sync.dma_start(
                o_out[sb * 128:(sb + 1) * 128,
                      ht * 512:(ht + 1) * 512], ob[:])

        with tc.tile_pool(name="attnsb", bufs=1) as asb:
          with tc.tile_pool(name="attnpsum", bufs=1, space="PSUM") as ap_:
            gcount = 0
            for c in range(NCHUNK):
                for h in range(NH_CORE):
                    for qt in range(NQT):
                        q0 = qt * QT
                        # z accumulator: one psum bank reused serially across
                        # qt (matmul psum writes at partition offsets != 0
                        # miscompute on HW, so everything lives at row 0)
                        pzs = ap_.tile([1, 512], F32, tag="z", bufs=1,
                                       name=f"pz{c}{h}{qt}")
                        # item: (qsrc_ap, ka, off, mask(moff, mw, bstart))
                        fulls = []
                        partials = []
                        nkb_i = min((q0 + QT + 127) // 128, NKB)
                        for kb in range(nkb_i):
                            off = max(0, kb * 128 - q0)
                            delta = q0 - kb * 128
                            mask = None
                            if delta < 128:
                                mw = min(128, QT - off)
                                mask = (off, mw, 512 + delta + off)
                            it = (qint[h][:, c * CL + q0:c * CL + q0 + QT],
                                  c * CL + kb * 128, off, mask)
                            (partials if off else fulls).append(it)
                        if c >= 1:
                            qs = qcrs[h][:, (c - 1) * CL + q0:
                                         (c - 1) * CL + q0 + QT]
                            for kb in range(NKB):
                                fulls.append((qs, (c - 1) * CL + kb * 128,
                                              0, None))
                        if c == 2:
                            qs = qfar[h][:, q0:q0 + QT]
                            for kb in range(NKB):
                                fulls.append((qs, kb * 128, 0, None))
                        # spread the masked partial blocks among the fulls so
                        # a low-work masked group never clusters at qt end
                        items = []
                        step = max(2, len(fulls) // (len(partials) + 1))
                        pi = 0
                        for i, it in enumerate(fulls):
                            items.append(it)
                            if (i + 1) % step == 0 and pi < len(partials):
                                items.append(partials[pi])
                                pi += 1
                        items += partials[pi:]
                        # pair consecutive full-width items
                        groups = []
                        i = 0
                        while i < len(items):
                            if (i + 1 < len(items) and items[i][2] == 0
                                    and items[i + 1][2] == 0):
                                groups.append((items[i], items[i + 1]))
                                i += 2
                            else:
                                groups.append((items[i],))
                                i += 1
                        npv = len(items)
                        pos_t = ap_.tile([128, 512], F32, tag="o", bufs=1,
                                         name=f"po{c}{h}{qt}")
                        cnt = [0]
                        pend = []      # software pipeline: PV/z lag QK by 2

                        def flush_one():
                            PT_, grp_ = pend.pop(0)
                            for sl, (qs, ka, off, mask) in enumerate(grp_):
                                cnt[0] += 1
                                fst = cnt[0] == 1
                                lst = cnt[0] == npv
                                nc.tensor.matmul(pos_t[:, off:QT],
                                                 v_sb[:, ka // 128, :],
                                                 PT_[:, sl, off:QT],
                                                 start=fst, stop=lst)
                                nc.tensor.matmul(
                                    pzs[0:1, off:QT],
                                    ones[:], PT_[:, sl, off:QT],
                                    start=fst, stop=lst)

                        for gi, grp in enumerate(groups):
                            ps_t = ap_.tile([128, 2, 512], F32, tag="s",
                                            bufs=2, name=f"ps{c}{h}{qt}_{gi}")
                            PT = asb.tile([128, 2, QT], FP16, tag="PT",
                                          bufs=4, name=f"PT{c}{h}{qt}_{gi}")
                            for sl, (qs, ka, off, mask) in enumerate(grp):
                                nc.tensor.matmul(ps_t[:, sl, off:QT],
                                                 kT[:, ka:ka + 128],
                                                 qs[:, off:QT],
                                                 start=True, stop=True)
                            if len(grp) == 2:
                                nc.scalar.activation(
                                    PT[:, :, :], ps_t[:, :, 0:QT],
                                    mybir.ActivationFunctionType.Exp,
                                    scale=SCALE)
                            else:
                                off = grp[0][2]
                                nc.scalar.activation(
                                    PT[:, 0, off:QT], ps_t[:, 0, off:QT],
                                    mybir.ActivationFunctionType.Exp,
                                    scale=SCALE)
                            for sl, (qs, ka, off, mask) in enumerate(grp):
                                if mask is not None:
                                    moff, mw, bs = mask
                                    nc.gpsimd.tensor_mul(
                                        PT[:, sl, moff:moff + mw],
                                        PT[:, sl, moff:moff + mw],
                                        bigtri[:, bs:bs + mw])
                            pend.append((PT, grp))
                            if len(pend) > 2:
                                flush_one()
                            gcount += 1
                            if gcount % 2 == 0 and oproj_q:
                                emit_oproj(ap_, 2, *oproj_q.pop(0))
                        while pend:
                            flush_one()
                        # normalize: rz = 1/z, broadcast, fused mul into attnT
                        rzf = asb.tile([1, QT], F32, tag="rzf", bufs=2,
                                       name=f"rzf{c}{h}{qt}")
                        nc.vector.reciprocal_approx_fast(
                            rzf[0:1, :], pzs[0:1, 0:QT])
                        rb = asb.tile([128, QT], F32, tag="rb", bufs=2,
                                      name=f"rb{c}{h}{qt}")
                        nc.gpsimd.partition_broadcast(rb[:], rzf[0:1, :])
                        nc.vector.tensor_mul(
                            attnT[h][:, c * CL + q0:c * CL + q0 + QT],
                            pos_t[:, 0:QT], rb[:])
                # chunk done for both heads: queue its o_proj units
                for sb in range(c * NKB, (c + 1) * NKB):
                    for ht in range(4):
                        oproj_q.append((sb, ht))
          # tail flush in a fresh psum pool (attention banks are free now,
          # so deeper buffering keeps the PE ahead of the copies)
          with tc.tile_pool(name="tailpsum", bufs=1, space="PSUM") as tp2:
            while oproj_q:
                emit_oproj(tp2, 5, *oproj_q.pop(0))
    nc.compile()
    return nc


def _sflip(sT):
    out = np.array(sT, dtype=np.float32)
    out[0:64] = -out[0:64]
    return out


def _prep_in_maps(inputs):
    f32 = np.float32
    hs = np.asarray(inputs["hidden_states"], f32).reshape(SEQ, HID)
    pos = np.asarray(inputs["position_ids"]).reshape(SEQ).astype(np.int64)
    pid = pos % CL
    q_cos = np.asarray(inputs["q_cos"], f32)
    q_sin = np.asarray(inputs["q_sin"], f32)
    qc_cos = np.asarray(inputs["qc_cos"], f32)
    qc_sin = np.asarray(inputs["qc_sin"], f32)
    k_cos = np.asarray(inputs["k_cos"], f32)
    k_sin = np.asarray(inputs["k_sin"], f32)
    Wq = np.asarray(inputs["Wq"], f32)
    Wk = np.asarray(inputs["Wk"], f32)
    Wv = np.asarray(inputs["Wv"], f32)
    Wo = np.asarray(inputs["Wo"], f32)

    hT = np.ascontiguousarray(hs.T).astype(NPBF16)
    cosI = np.ascontiguousarray(q_cos[pid].T).astype(NPBF16)
    sinIS = _sflip(q_sin[pid].T).astype(NPBF16)
    # cross tables for chunks 1..2 (columns (c-1)*CL..c*CL map to chunk c)
    cosC = np.ascontiguousarray(qc_cos[pid[CL:3 * CL]].T).astype(NPBF16)
    sinCS = _sflip(qc_sin[pid[CL:3 * CL]].T).astype(NPBF16)
    cosF = np.ascontiguousarray(qc_cos[CL - 1][:, None]).astype(f32)
    sinFS = _sflip(qc_sin[CL - 1][:, None]).astype(f32)
    cosK = np.ascontiguousarray(k_cos[pos].T).astype(NPBF16)
    sinKS = _sflip(k_sin[pos].T).astype(NPBF16)
    bigtri = (np.arange(128)[:, None] <= (np.arange(928)[None, :] - 512)
              ).astype(np.float16)

    shared = dict(hT=hT, cosI=cosI, sinIS=sinIS, cosC=cosC, sinCS=sinCS,
                  cosF=cosF, sinFS=sinFS, cosK=cosK, sinKS=sinKS,
                  bigtri=bigtri)
    in_maps = []
    for core in range(N_CORES):
        kv = core // 2
        m = dict(shared)
        m["wq"] = np.ascontiguousarray(
            Wq[256 * core:256 * (core + 1), :].T).astype(NPBF16)
        m["wk"] = np.ascontiguousarray(
            Wk[128 * kv:128 * (kv + 1), :].T).astype(NPBF16)
        m["wv"] = np.ascontiguousarray(
            Wv[128 * kv:128 * (kv + 1), :].T).astype(NPBF16)
        m["wo"] = np.ascontiguousarray(
            Wo[:, 256 * core:256 * (core + 1)].T).astype(NPBF16)
        in_maps.append(m)
    return in_maps


_CACHE = {}


def _get_nc():
    if "nc" not in _CACHE:
        _CACHE["nc"] = _build()
    return _CACHE["nc"]


def kernel(**inputs):
    nc = _get_nc()
    in_maps = _prep_in_maps(inputs)
    res = run_bass_kernel_spmd(nc, in_maps, list(range(N_CORES)))
    out = np.zeros((SEQ, HID), np.float32)
    for r in res.results:
        out += r["o_out"].astype(np.float32)
    return out.reshape(1, SEQ, HID).astype(np.float32)


# revision 21
# speedup vs baseline: 1.4666x; 1.2857x over previous
# Trainium2 Bass kernel for ChunkLlamaAttention (chunked attention w/ 3 rope
# variants + LSE merge), tensor-parallel over 8 NeuronCores.
#
# Sharding: 16 q-heads / 4 kv-heads split as 2 q-heads + 1 kv-head per core.
# Each core: QKV projections (bf16 matmuls, f32 psum) -> fused k-rope ->
# 3 roped copies of q (intra / cross / far) -> unified-softmax chunked
# attention (the reference's per-part LSE merge == one softmax over the
# union of keys, with q roped per key-block's chunk distance) -> o_proj
# partial (columns of attn heads x Wo^T rows). Host sums the 8 partials.
#
# Layouts: q/k kept transposed [head_dim(128 part), seq]; scores computed as
# S^T [k, q] so softmax denom comes from a ones-stationary matmul; PV uses
# v [k, d] stationary giving out^T [d, q] which feeds o_proj stationary
# directly. exp() runs on ACT (scale folded in); probabilities in fp16.
#
# Scheduling (v1 restructure):
#  - DMA order: proj-critical tensors first; rope tables mid proj loop.
#  - v-transposes + per-chunk q-rope interleaved into the proj loop so
#    attention starts right at proj end.
#  - Attention per (c,h,qt): k-blocks paired so one ACT exp covers two
#    score tiles (2-bank [128,2,512] psum tiles); intra diagonal blocks
#    column-trimmed; triangular masks on the (idle) Pool engine; the 4
#    qt softmax denominators accumulate in one psum bank at partition
#    offsets 0/32/64/96; fast approx reciprocal + partition broadcast +
#    fused normalize-copy (psum -> attnT in a single DVE multiply).
#  - o_proj of chunk c interleaved into chunk c+1's attention; tail
#    copies alternate ACT/DVE.
import numpy as np
import ml_dtypes
from contextlib import ExitStack

import concourse.bass as bass
import concourse.mybir as mybir
import concourse.tile as tile
from concourse import bacc
from concourse.bass_utils import run_bass_kernel_spmd
from concourse.masks import make_identity

BF16 = mybir.dt.bfloat16
FP16 = mybir.dt.float16
F32 = mybir.dt.float32
NPBF16 = ml_dtypes.bfloat16

N_CORES = 8
SEQ = 4992
HID = 2048
CL = 1664          # chunk length
NCHUNK = 3
D = 128            # head dim
NH_CORE = 2        # q heads per core
QT = 416           # q tile (4 per chunk)
NQT = 4
NKB = CL // 128    # 13 k-blocks per chunk
HC = HID // 128    # 16 hidden chunks
NSB = SEQ // 128   # 39 s-blocks
SCALE = float(D) ** -0.5


def _build():
    nc = bacc.Bacc("TRN2", target_bir_lowering=False, debug=False,
                   num_devices=N_CORES)
    hT = nc.dram_tensor("hT", [HID, SEQ], BF16, kind="ExternalInput").ap()
    wq = nc.dram_tensor("wq", [HID, NH_CORE * D], BF16, kind="ExternalInput").ap()
    wk = nc.dram_tensor("wk", [HID, D], BF16, kind="ExternalInput").ap()
    wv = nc.dram_tensor("wv", [HID, D], BF16, kind="ExternalInput").ap()
    wo = nc.dram_tensor("wo", [NH_CORE * D, HID], BF16, kind="ExternalInput").ap()
    cosI = nc.dram_tensor("cosI", [D, SEQ], BF16, kind="ExternalInput").ap()
    sinIS = nc.dram_tensor("sinIS", [D, SEQ], BF16, kind="ExternalInput").ap()
    cosC = nc.dram_tensor("cosC", [D, 2 * CL], BF16, kind="ExternalInput").ap()
    sinCS = nc.dram_tensor("sinCS", [D, 2 * CL], BF16, kind="ExternalInput").ap()
    cosF = nc.dram_tensor("cosF", [D, 1], F32, kind="ExternalInput").ap()
    sinFS = nc.dram_tensor("sinFS", [D, 1], F32, kind="ExternalInput").ap()
    cosK = nc.dram_tensor("cosK", [D, SEQ], BF16, kind="ExternalInput").ap()
    sinKS = nc.dram_tensor("sinKS", [D, SEQ], BF16, kind="ExternalInput").ap()
    bigtri_in = nc.dram_tensor("bigtri", [D, 928], FP16, kind="ExternalInput").ap()
    o_out = nc.dram_tensor("o_out", [SEQ, HID], BF16, kind="ExternalOutput").ap()

    with tile.TileContext(nc) as tc, ExitStack() as ctx:
        persist = ctx.enter_context(tc.tile_pool(name="persist", bufs=1))
        ones = persist.tile([128, 1], FP16)
        nc.gpsimd.memset(ones[:], 1.0)
        ones_row = persist.tile([1, 128], F32)
        nc.gpsimd.memset(ones_row[:], 1.0)
        ident = persist.tile([128, 128], FP16)
        make_identity(nc, ident[:])

        # proj-critical DMAs first: weight slices, then k-rope tables
        wq_sb = persist.tile([128, HC, NH_CORE * D], BF16)
        nc.sync.dma_start(wq_sb[:], wq.rearrange("(hc p) d -> p hc d", p=128))
        wk_sb = persist.tile([128, HC, D], BF16)
        nc.sync.dma_start(wk_sb[:], wk.rearrange("(hc p) d -> p hc d", p=128))
        wv_sb = persist.tile([128, HC, D], BF16)
        nc.sync.dma_start(wv_sb[:], wv.rearrange("(hc p) d -> p hc d", p=128))

        kT = persist.tile([128, SEQ], BF16)          # roped keys [d, s]
        v_sb = persist.tile([128, NSB, 128], FP16)   # [s_in_blk, blk, d]
        attnT = [persist.tile([128, SEQ], BF16, name=f"attnT{h}")
                 for h in range(NH_CORE)]
        wo_sb = persist.tile([128, NH_CORE, HID], BF16)
        bigtri = persist.tile([128, 928], FP16)
        cosF_sb = persist.tile([128, 1], F32)
        sinFS_sb = persist.tile([128, 1], F32)

        qraw_pool = ctx.enter_context(tc.tile_pool(name="qraw", bufs=1))
        qraw = [qraw_pool.tile([128, SEQ], BF16, name=f"qraw{h}")
                for h in range(NH_CORE)]
        rp = ctx.enter_context(tc.tile_pool(name="ropesb", bufs=1))
        ct = rp.tile([128, SEQ], BF16, tag="ct", name="cosI_sb")
        st_t = rp.tile([128, SEQ], BF16, tag="st", name="sinIS_sb")
        ct2 = rp.tile([128, 2 * CL], BF16, tag="ct2", name="cosC_sb")
        st2 = rp.tile([128, 2 * CL], BF16, tag="st2", name="sinCS_sb")

        qsets = ctx.enter_context(tc.tile_pool(name="qsets", bufs=1))
        qint = [qsets.tile([128, SEQ], BF16, name=f"qint{h}")
                for h in range(NH_CORE)]
        qcrs = [qsets.tile([128, 2 * CL], BF16, name=f"qcrs{h}")
                for h in range(NH_CORE)]
        qfar = [qsets.tile([128, CL], BF16, name=f"qfar{h}")
                for h in range(NH_CORE)]

        def rope_block(dst, src_ap, ct_ap, st_ap, L, nm):
            # dst = src*cos + rot_half(src)*sin_signed, all on DVE
            m = rp.tile([128, CL], BF16, tag="ropem", bufs=1, name=f"m{nm}")
            r = rp.tile([128, CL], BF16, tag="roper", bufs=1, name=f"r{nm}")
            t = rp.tile([128, CL], BF16, tag="ropet", bufs=1, name=f"t{nm}")
            nc.vector.tensor_copy(r[0:64, 0:L], src_ap[64:128])
            nc.vector.tensor_copy(r[64:128, 0:L], src_ap[0:64])
            nc.vector.tensor_mul(m[:, 0:L], src_ap, ct_ap)
            nc.vector.tensor_mul(t[:, 0:L], r[:, 0:L], st_ap)
            nc.vector.tensor_add(dst, m[:, 0:L], t[:, 0:L])

        def rope_chunk(c):
            # intra rope for chunk c; cross for c>=1; far for c==2
            a, b = c * CL, (c + 1) * CL
            for h in range(NH_CORE):
                rope_block(qint[h][:, a:b], qraw[h][:, a:b], ct[:, a:b],
                           st_t[:, a:b], CL, f"i{h}{c}")
            if c >= 1:
                ca, cb = (c - 1) * CL, c * CL
                for h in range(NH_CORE):
                    rope_block(qcrs[h][:, ca:cb], qraw[h][:, a:b],
                               ct2[:, ca:cb], st2[:, ca:cb], CL, f"c{h}{c}")
            if c == 2:
                for h in range(NH_CORE):
                    m = rp.tile([128, CL], BF16, tag="ropem", bufs=1,
                                name=f"mf{h}")
                    r = rp.tile([128, CL], BF16, tag="roper", bufs=1,
                                name=f"rf{h}")
                    nc.vector.tensor_copy(r[0:64, :], qraw[h][64:128, a:b])
                    nc.vector.tensor_copy(r[64:128, :], qraw[h][0:64, a:b])
                    nc.vector.tensor_scalar_mul(m[:], qraw[h][:, a:b],
                                                cosF_sb[:])
                    nc.vector.scalar_tensor_tensor(
                        qfar[h][:, :], r[:], sinFS_sb[:], m[:],
                        op0=mybir.AluOpType.mult, op1=mybir.AluOpType.add)

        # ---------- Phase 1: QKV projections + fused k-rope, interleaved
        # with v-transpose per s-tile, q-rope per finished chunk, and the
        # deferred DMAs ----------
        with tc.tile_pool(name="projsb", bufs=1) as pj, \
             tc.tile_pool(name="projpsum", bufs=1, space="PSUM") as pp, \
             tc.tile_pool(name="tpsum", bufs=2, space="PSUM") as tp:
            cosK_sb = pj.tile([128, SEQ], BF16)
            nc.sync.dma_start(cosK_sb[:], cosK[:])
            sinKS_sb = pj.tile([128, SEQ], BF16)
            nc.sync.dma_start(sinKS_sb[:], sinKS[:])
            n_st = (SEQ + 511) // 512
            for st in range(n_st):
                s0 = st * 512
                L = min(512, SEQ - s0)
                hts = []
                for hc in range(HC):
                    ht_t = pj.tile([128, 512], BF16, tag="htile", bufs=16,
                                   name=f"ht_{st}_{hc}")
                    nc.sync.dma_start(ht_t[:, 0:L], hT[hc * 128:(hc + 1) * 128,
                                                       s0:s0 + L])
                    hts.append(ht_t)
                pq0 = pp.tile([128, 512], F32, tag="pq0")
                pq1 = pp.tile([128, 512], F32, tag="pq1")
                pk = pp.tile([128, 512], F32, tag="pk", bufs=2)
                pv = pp.tile([128, 512], F32, tag="pv")
                for hc in range(HC):
                    fst = hc == 0
                    lst = hc == HC - 1
                    rhs = hts[hc][:, 0:L]
                    nc.tensor.matmul(pq0[:, 0:L], wq_sb[:, hc, 0:128], rhs,
                                     start=fst, stop=lst)
                    nc.tensor.matmul(pq1[:, 0:L], wq_sb[:, hc, 128:256], rhs,
                                     start=fst, stop=lst)
                    nc.tensor.matmul(pk[:, 0:L], wk_sb[:, hc, :], rhs,
                                     start=fst, stop=lst)
                    nc.tensor.matmul(pv[:, 0:L], wv_sb[:, hc, :], rhs,
                                     start=fst, stop=lst)
                nc.vector.tensor_copy(qraw[0][:, s0:s0 + L], pq0[:, 0:L])
                nc.vector.tensor_copy(qraw[1][:, s0:s0 + L], pq1[:, 0:L])
                vT = pj.tile([128, 512], FP16, tag="vT", bufs=2,
                             name=f"vT{st}")
                nc.scalar.copy(vT[:, 0:L], pv[:, 0:L])
                # fused k rope: kT = pk*cosK + rot(pk)*sinKS
                # (walrus requires equal input base partitions for 2-input
                # DVE ops, so stage the half-swap via 1-input copies)
                mt = pj.tile([128, 512], BF16, tag="mt", bufs=1)
                rt = pj.tile([128, 512], BF16, tag="rt", bufs=1)
                tt = pj.tile([128, 512], BF16, tag="tt", bufs=1)
                nc.vector.tensor_copy(rt[0:64, 0:L], pk[64:128, 0:L])
                nc.vector.tensor_copy(rt[64:128, 0:L], pk[0:64, 0:L])
                nc.vector.tensor_mul(mt[:, 0:L], pk[:, 0:L],
                                     cosK_sb[:, s0:s0 + L])
                nc.vector.tensor_mul(tt[:, 0:L], rt[:, 0:L],
                                     sinKS_sb[:, s0:s0 + L])
                nc.vector.tensor_add(kT[:, s0:s0 + L], mt[:, 0:L], tt[:, 0:L])
                # v transpose for the s-blocks this tile covers
                for sb in range(s0 // 128, (s0 + L) // 128):
                    w0 = sb * 128 - s0
                    ptr = tp.tile([128, 128], FP16, tag="ptr")
                    nc.tensor.transpose(ptr[:], vT[:, w0:w0 + 128], ident[:])
                    nc.scalar.copy(v_sb[:, sb, :], ptr[:])
                # deferred DMAs + per-chunk rope, injected at milestones
                if st == 0:
                    nc.sync.dma_start(ct[:], cosI[:])
                    nc.sync.dma_start(st_t[:], sinIS[:])
                elif st == 1:
                    nc.sync.dma_start(ct2[:], cosC[:])
                    nc.sync.dma_start(st2[:], sinCS[:])
                elif st == 2:
                    nc.sync.dma_start(bigtri[:], bigtri_in[:])
                    nc.sync.dma_start(cosF_sb[:], cosF[:])
                    nc.sync.dma_start(sinFS_sb[:], sinFS[:])
                    nc.sync.dma_start(
                        wo_sb[:], wo.rearrange("(fc p) h -> p fc h", p=128))
                elif st == 3:
                    rope_chunk(0)
                elif st == 6:
                    rope_chunk(1)
                elif st == 9:
                    rope_chunk(2)

        # ---------- Phase 2: attention per chunk + interleaved o_proj ----
        # Per (c, h, qt): iterate k-blocks (intra column-trimmed), paired so
        # one exp covers two score tiles. PV accumulates pos [d, qt]; the
        # denominator z accumulates in pzs partition row 32*qt. Then fast
        # reciprocal -> broadcast -> fused normalize into attnT.
        oproj_q = []          # pending o_proj units from the previous chunk
        eng_alt = [0]

        def emit_oproj(pool, bufs, sb, ht):
            pp_ = pool.tile([128, 512], F32, tag="po_", bufs=bufs,
                            name=f"pp{sb}{ht}")
            for fc in range(NH_CORE):
                nc.tensor.matmul(
                    pp_[:],
                    attnT[fc][:, sb * 128:(sb + 1) * 128],
                    wo_sb[:, fc, ht * 512:(ht + 1) * 512],
                    start=(fc == 0), stop=(fc == NH_CORE - 1))
            ob = asb.tile([128, 512], BF16, tag="ob", bufs=4,
                          name=f"ob{sb}{ht}")
            eng_alt[0] += 1
            if eng_alt[0] % 2 == 0:
                nc.vector.tensor_copy(ob[:], pp_[:])
            else:
                nc.scalar.copy(ob[:], pp_[:])
            nc.sync.dma_start(
                o_out[sb * 128:(sb + 1) * 128,
                      ht * 512:(ht + 1) * 512], ob[:])

        with tc.tile_pool(name="attnsb", bufs=1) as asb:
          with tc.tile_pool(name="attnpsum", bufs=1, space="PSUM") as ap_:
            gcount = 0
            pending_norm = []

            def flush_norm():
                # deferred normalize: rb = ones_col x rz (PE matmul broadcast
                # into the consumed z bank), then in-place scale of attnT.
                # Deferred into the NEXT qt's stream so the PE does not sit
                # on the reciprocal, and emitted before that qt's first z
                # flush (same psum buffer) to keep the engines deadlock-free.
                pzs_, rzf_, dst = pending_norm.pop(0)
                nc.tensor.matmul(pzs_[0:128, 0:QT], ones_row[:], rzf_[0:1, :],
                                 start=True, stop=True)
                nc.vector.tensor_mul(dst, dst, pzs_[0:128, 0:QT])

            for c in range(NCHUNK):
                for h in range(NH_CORE):
                    for qt in range(NQT):
                        q0 = qt * QT
                        # z accumulator in row 0 of a full psum bank (matmul
                        # psum writes at partition offsets != 0 miscompute on
                        # HW); rows 0..127 later reused for the rb broadcast
                        pzs = ap_.tile([128, 512], F32, tag="z", bufs=1,
                                       name=f"pz{c}{h}{qt}")
                        # item: (qsrc_ap, ka, off, mask(moff, mw, bstart))
                        fulls = []
                        partials = []
                        nkb_i = min((q0 + QT + 127) // 128, NKB)
                        for kb in range(nkb_i):
                            off = max(0, kb * 128 - q0)
                            delta = q0 - kb * 128
                            mask = None
                            if delta < 128:
                                mw = min(128, QT - off)
                                mask = (off, mw, 512 + delta + off)
                            it = (qint[h][:, c * CL + q0:c * CL + q0 + QT],
                                  c * CL + kb * 128, off, mask)
                            (partials if off else fulls).append(it)
                        if c >= 1:
                            qs = qcrs[h][:, (c - 1) * CL + q0:
                                         (c - 1) * CL + q0 + QT]
                            for kb in range(NKB):
                                fulls.append((qs, (c - 1) * CL + kb * 128,
                                              0, None))
                        if c == 2:
                            qs = qfar[h][:, q0:q0 + QT]
                            for kb in range(NKB):
                                fulls.append((qs, kb * 128, 0, None))
                        # spread the masked partial blocks among the fulls so
                        # a low-work masked group never clusters at qt end
                        items = []
                        step = max(2, len(fulls) // (len(partials) + 1))
                        pi = 0
                        for i, it in enumerate(fulls):
                            items.append(it)
                            if (i + 1) % step == 0 and pi < len(partials):
                                items.append(partials[pi])
                                pi += 1
                        items += partials[pi:]
                        # pair consecutive full-width items
                        groups = []
                        i = 0
                        while i < len(items):
                            if (i + 1 < len(items) and items[i][2] == 0
                                    and items[i + 1][2] == 0):
                                groups.append((items[i], items[i + 1]))
                                i += 2
                            else:
                                groups.append((items[i],))
                                i += 1
                        npv = len(items)
                        pos_t = ap_.tile([128, 512], F32, tag="o", bufs=1,
                                         name=f"po{c}{h}{qt}")
                        cnt = [0]
                        pend = []      # software pipeline: PV/z lag QK by 2

                        def flush_one():
                            PT_, grp_ = pend.pop(0)
                            for sl, (qs, ka, off, mask) in enumerate(grp_):
                                cnt[0] += 1
                                fst = cnt[0] == 1
                                lst = cnt[0] == npv
                                nc.tensor.matmul(pos_t[:, off:QT],
                                                 v_sb[:, ka // 128, :],
                                                 PT_[:, sl, off:QT],
                                                 start=fst, stop=lst)
                                nc.tensor.matmul(
                                    pzs[0:1, off:QT],
                                    ones[:], PT_[:, sl, off:QT],
                                    start=fst, stop=lst)

                        for gi, grp in enumerate(groups):
                            ps_t = ap_.tile([128, 2, 512], F32, tag="s",
                                            bufs=2, name=f"ps{c}{h}{qt}_{gi}")
                            PT = asb.tile([128, 2, QT], FP16, tag="PT",
                                          bufs=4, name=f"PT{c}{h}{qt}_{gi}")
                            for sl, (qs, ka, off, mask) in enumerate(grp):
                                nc.tensor.matmul(ps_t[:, sl, off:QT],
                                                 kT[:, ka:ka + 128],
                                                 qs[:, off:QT],
                                                 start=True, stop=True)
                            if len(grp) == 2:
                                nc.scalar.activation(
                                    PT[:, :, :], ps_t[:, :, 0:QT],
                                    mybir.ActivationFunctionType.Exp,
                                    scale=SCALE)
                            else:
                                off = grp[0][2]
                                nc.scalar.activation(
                                    PT[:, 0, off:QT], ps_t[:, 0, off:QT],
                                    mybir.ActivationFunctionType.Exp,
                                    scale=SCALE)
                            for sl, (qs, ka, off, mask) in enumerate(grp):
                                if mask is not None:
                                    moff, mw, bs = mask
                                    nc.gpsimd.tensor_mul(
                                        PT[:, sl, moff:moff + mw],
                                        PT[:, sl, moff:moff + mw],
                                        bigtri[:, bs:bs + mw])
                            pend.append((PT, grp))
                            if len(pend) > 2:
                                flush_one()
                            if gi == 1 and pending_norm:
                                flush_norm()
                            gcount += 1
                            if gcount % 2 == 0 and oproj_q:
                                emit_oproj(ap_, 2, *oproj_q.pop(0))
                        while pend:
                            flush_one()
                        # unnormalized copy out, then 1/z; the broadcast and
                        # in-place scale are deferred into the next qt
                        dst = attnT[h][:, c * CL + q0:c * CL + q0 + QT]
                        nc.vector.tensor_copy(dst, pos_t[:, 0:QT])
                        rzf = asb.tile([1, QT], F32, tag="rzf", bufs=2,
                                       name=f"rzf{c}{h}{qt}")
                        nc.vector.reciprocal_approx_fast(
                            rzf[0:1, :], pzs[0:1, 0:QT])
                        pending_norm.append((pzs, rzf, dst))
                # chunk done for both heads: queue its o_proj units
                for sb in range(c * NKB, (c + 1) * NKB):
                    for ht in range(4):
                        oproj_q.append((sb, ht))
            while pending_norm:
                flush_norm()
          # tail flush in a fresh psum pool (attention banks are free now,
          # so deeper buffering keeps the PE ahead of the copies)
          with tc.tile_pool(name="tailpsum", bufs=1, space="PSUM") as tp2:
            while oproj_q:
                emit_oproj(tp2, 5, *oproj_q.pop(0))
    nc.compile()
    return nc


def _sflip(sT):
    out = np.array(sT, dtype=np.float32)
    out[0:64] = -out[0:64]
    return out


def _prep_in_maps(inputs):
    f32 = np.float32
    hs = np.asarray(inputs["hidden_states"], f32).reshape(SEQ, HID)
    pos = np.asarray(inputs["position_ids"]).reshape(SEQ).astype(np.int64)
    pid = pos % CL
    q_cos = np.asarray(inputs["q_cos"], f32)
    q_sin = np.asarray(inputs["q_sin"], f32)
    qc_cos = np.asarray(inputs["qc_cos"], f32)
    qc_sin = np.asarray(inputs["qc_sin"], f32)
    k_cos = np.asarray(inputs["k_cos"], f32)
    k_sin = np.asarray(inputs["k_sin"], f32)
    Wq = np.asarray(inputs["Wq"], f32)
    Wk = np.asarray(inputs["Wk"], f32)
    Wv = np.asarray(inputs["Wv"], f32)
    Wo = np.asarray(inputs["Wo"], f32)

    hT = np.ascontiguousarray(hs.T).astype(NPBF16)
    cosI = np.ascontiguousarray(q_cos[pid].T).astype(NPBF16)
    sinIS = _sflip(q_sin[pid].T).astype(NPBF16)
    # cross tables for chunks 1..2 (columns (c-1)*CL..c*CL map to chunk c)
    cosC = np.ascontiguousarray(qc_cos[pid[CL:3 * CL]].T).astype(NPBF16)
    sinCS = _sflip(qc_sin[pid[CL:3 * CL]].T).astype(NPBF16)
    cosF = np.ascontiguousarray(qc_cos[CL - 1][:, None]).astype(f32)
    sinFS = _sflip(qc_sin[CL - 1][:, None]).astype(f32)
    cosK = np.ascontiguousarray(k_cos[pos].T).astype(NPBF16)
    sinKS = _sflip(k_sin[pos].T).astype(NPBF16)
    bigtri = (np.arange(128)[:, None] <= (np.arange(928)[None, :] - 512)
              ).astype(np.float16)

    shared = dict(hT=hT, cosI=cosI, sinIS=sinIS, cosC=cosC, sinCS=sinCS,
                  cosF=cosF, sinFS=sinFS, cosK=cosK, sinKS=sinKS,
                  bigtri=bigtri)
    in_maps = []
    for core in range(N_CORES):
        kv = core // 2
        m = dict(shared)
        m["wq"] = np.ascontiguousarray(
            Wq[256 * core:256 * (core + 1), :].T).astype(NPBF16)
        m["wk"] = np.ascontiguousarray(
            Wk[128 * kv:128 * (kv + 1), :].T).astype(NPBF16)
        m["wv"] = np.ascontiguousarray(
            Wv[128 * kv:128 * (kv + 1), :].T).astype(NPBF16)
        m["wo"] = np.ascontiguousarray(
            Wo[:, 256 * core:256 * (core + 1)].T).astype(NPBF16)
        in_maps.append(m)
    return in_maps


_CACHE = {}


def _get_nc():
    if "nc" not in _CACHE:
        _CACHE["nc"] = _build()
    return _CACHE["nc"]


def kernel(**inputs):
    nc = _get_nc()
    in_maps = _prep_in_maps(inputs)
    res = run_bass_kernel_spmd(nc, in_maps, list(range(N_CORES)))
    out = np.zeros((SEQ, HID), np.float32)
    for r in res.results:
        out += r["o_out"].astype(np.float32)
    return out.reshape(1, SEQ, HID).astype(np.float32)
